# revision 1
# baseline (speedup 1.0000x reference)
"""Trainium2 Bass kernel for nn_Attention_80805514707533.

Recurrent attention scan: B=512, T=512, C=64, H=128.
Sharding: H across 8 cores (16 heads each); full batch B=512 rides the
matmul moving dimension. C=64 lives on partitions; heads are packed in
pairs (2 x 64 = 128 partitions) with block-diagonal stationary weights.

Per step t (per core, j = head-pair 0..7):
  pre[j]  = Wi_cat[j].T @ xT_t  +  Wa_blk[j].T @ att[:,j,:]      (PSUM)
  v[j]    = tanh(pre[j])                                          (ACT)
  e[j]    = We_blk[j].T @ v[j]                                    (PSUM)
  u[j]    = exp(e[j])                                             (ACT)
  w       = u * xdup  (broadcast over j)                          (GPSIMD)
  S       = sum_c u   via ones16 matmuls  -> [16, B]              (PE)
  num     = sum_c u*x via ones16 matmuls  -> [16, B]              (PE)
  rS      = 1/S                                                   (DVE)
  out_t   = num * rS  -> DMA to DRAM                              (DVE)
  att'    = u * bcast(rS)   (bcast via selector matmul)           (PE+DVE)
"""

import os
import numpy as np

B, T, C, H = 512, 512, 64, 128
NCORES = 8
HL = H // NCORES          # heads per core = 16
NPAIR = HL // 2           # head pairs per core = 8
FP = None                 # set lazily (mybir.dt.float32)


def _build_nc(t_steps: int, abl=frozenset()):
    import concourse.bass as bass
    import concourse.bacc as bacc
    import concourse.mybir as mybir
    import concourse.tile as tile
    from contextlib import ExitStack

    fp32 = mybir.dt.float32
    fp32r = mybir.dt.float32r
    r = lambda ap: ap.bitcast(fp32r)
    nc = bacc.Bacc("TRN2", target_bir_lowering=False, debug=False,
                   num_devices=NCORES)

    xT_d = nc.dram_tensor("xT", [C, t_steps, B], fp32r, kind="ExternalInput")
    wi_d = nc.dram_tensor("wi", [C, NPAIR, 128], fp32r, kind="ExternalInput")
    wa_d = nc.dram_tensor("wa", [128, NPAIR, 128], fp32r, kind="ExternalInput")
    we_d = nc.dram_tensor("we", [128, NPAIR, 128], fp32r, kind="ExternalInput")
    on_d = nc.dram_tensor("ones16", [128, NPAIR, 48], fp32r, kind="ExternalInput")
    sel_d = nc.dram_tensor("sel", [HL, NPAIR, 128], fp32r, kind="ExternalInput")
    out_d = nc.dram_tensor("out", [t_steps, HL, B], fp32, kind="ExternalOutput")

    with ExitStack() as ctx:
        ctx.enter_context(nc.allow_low_precision(reason="fp32r matmul path"))
        tc = ctx.enter_context(tile.TileContext(nc))
        singles = ctx.enter_context(tc.tile_pool(name="singles", bufs=1))
        state = ctx.enter_context(tc.tile_pool(name="state", bufs=2))
        xpool = ctx.enter_context(tc.tile_pool(name="xpool", bufs=3))
        vpool = ctx.enter_context(tc.tile_pool(name="vpool", bufs=3 if ("nonorm" in abl and "nos3" in abl) else 2))
        upool = ctx.enter_context(tc.tile_pool(name="upool", bufs=3 if "nonorm" in abl else 2))
        wpool = ctx.enter_context(tc.tile_pool(name="wpool", bufs=2))
        spool = ctx.enter_context(tc.tile_pool(name="spool", bufs=3))
        opool = ctx.enter_context(tc.tile_pool(name="opool", bufs=3))
        ps_pre = ctx.enter_context(tc.tile_pool(name="ps_pre", bufs=2, space="PSUM"))
        ps_e = ctx.enter_context(tc.tile_pool(name="ps_e", bufs=2, space="PSUM"))
        ps_sn = ctx.enter_context(tc.tile_pool(name="ps_sn", bufs=1, space="PSUM"))
        ps_bc = ctx.enter_context(tc.tile_pool(name="ps_bc", bufs=2, space="PSUM"))

        wi_sb = singles.tile([C, NPAIR, 128], fp32)
        wa_sb = singles.tile([128, NPAIR, 128], fp32)
        we_sb = singles.tile([128, NPAIR, 128], fp32)
        on_sb = singles.tile([128, NPAIR, 48], fp32)
        sel_sb = singles.tile([HL, NPAIR, 128], fp32)
        nc.sync.dma_start(out=r(wi_sb), in_=wi_d[:])
        nc.sync.dma_start(out=r(wa_sb), in_=wa_d[:])
        nc.sync.dma_start(out=r(we_sb), in_=we_d[:])
        nc.sync.dma_start(out=r(on_sb), in_=on_d[:])
        nc.sync.dma_start(out=r(sel_sb), in_=sel_d[:])

        att = state.tile([128, NPAIR, B], fp32, tag="att")
        nc.vector.memset(att, 1.0 / C)

        for t in range(t_steps):
            xdup = xpool.tile([128, B], fp32)
            nc.sync.dma_start(out=r(xdup[0:C, :]), in_=xT_d[:, t, :])
            nc.sync.dma_start(out=r(xdup[C:128, :]), in_=xT_d[:, t, :])

            v_sb = vpool.tile([128, NPAIR, B], fp32)
            for j in range(NPAIR):
                pre = ps_pre.tile([128, B], fp32)
                nc.tensor.matmul(pre, r(wi_sb[:, j, :]), r(xdup[0:C, :]),
                                 start=True, stop=False)
                nc.tensor.matmul(pre, r(wa_sb[:, j, :]), r(att[:, j, :]),
                                 start=False, stop=True)
                nc.scalar.activation(r(v_sb[:, j, :]), pre,
                                     mybir.ActivationFunctionType.Tanh)

            if "nos3" in abl:
                u_sb = v_sb
            else:
                u_sb = upool.tile([128, NPAIR, B], fp32)
                for j in range(NPAIR):
                    e = ps_e.tile([128, B], fp32)
                    nc.tensor.matmul(e, r(we_sb[:, j, :]), r(v_sb[:, j, :]),
                                     start=True, stop=True)
                    nc.scalar.activation(r(u_sb[:, j, :]), e,
                                         mybir.ActivationFunctionType.Exp)

            # w = u * x  (per pair, so it pipelines behind each exp)
            if "noout" not in abl:
                w_sb = wpool.tile([128, NPAIR, B], fp32)
                for j in range(NPAIR):
                    nc.vector.tensor_mul(r(w_sb[:, j, :]), u_sb[:, j, :], xdup)

            if "nonorm" not in abl:
                S_ps = ps_sn.tile([HL, B], fp32, tag="S")
                for j in range(NPAIR):
                    nc.tensor.matmul(S_ps, r(on_sb[:, j, 0:HL]),
                                     r(u_sb[:, j, :]),
                                     start=(j == 0), stop=(j == NPAIR - 1))
            if "noout" not in abl:
                num_ps = ps_sn.tile([HL, B], fp32, tag="num")
                for j in range(NPAIR):
                    nc.tensor.matmul(num_ps, r(on_sb[:, j, 32:48]),
                                     r(w_sb[:, j, :]),
                                     start=(j == 0), stop=(j == NPAIR - 1))

            if "nonorm" not in abl:
                rS = spool.tile([HL, B], fp32)
                nc.vector.reciprocal(r(rS), S_ps)
            if "noout" not in abl:
                outb = opool.tile([HL, B], fp32)
                nc.vector.tensor_mul(outb, num_ps, rS)  # keep on DVE: gpsimd cannot read PSUM
                nc.sync.dma_start(out=out_d[t], in_=outb)

            # att' = u * bcast(rS) ; bcast via selector matmul per pair
            if "nonorm" not in abl:
                att_new = state.tile([128, NPAIR, B], fp32, tag="att")
                for j in range(NPAIR):
                    bc = ps_bc.tile([128, B], fp32)
                    nc.tensor.matmul(bc, r(sel_sb[:, j, :]), r(rS),
                                     start=True, stop=True)
                    nc.vector.tensor_mul(r(att_new[:, j, :]), u_sb[:, j, :], bc)
                att = att_new
            else:
                att = u_sb

    nc.compile()
    return nc


def _host_prep(x, weight_att, weight_input, weight_e):
    """Build per-core input maps (host-side layout prep)."""
    xT = np.ascontiguousarray(x.transpose(2, 1, 0))  # [C, T, B]

    in_maps = []
    for g in range(NCORES):
        h0 = g * HL
        wi = np.zeros((C, NPAIR, 128), np.float32)
        wa = np.zeros((128, NPAIR, 128), np.float32)
        we = np.zeros((128, NPAIR, 128), np.float32)
        on = np.zeros((128, NPAIR, 48), np.float32)
        sel = np.zeros((HL, NPAIR, 128), np.float32)
        for j in range(NPAIR):
            ha, hb = h0 + 2 * j, h0 + 2 * j + 1
            # lhsT[k, m] = W[h, m, k]
            wi[:, j, 0:C] = weight_input[ha].T
            wi[:, j, C:128] = weight_input[hb].T
            wa[0:C, j, 0:C] = weight_att[ha].T
            wa[C:128, j, C:128] = weight_att[hb].T
            we[0:C, j, 0:C] = weight_e[ha].T
            we[C:128, j, C:128] = weight_e[hb].T
            on[0:C, j, 2 * j] = 1.0
            on[C:128, j, 2 * j + 1] = 1.0
            on[0:C, j, 32 + 2 * j] = 1.0
            on[C:128, j, 32 + 2 * j + 1] = 1.0
            sel[2 * j, j, 0:C] = 1.0
            sel[2 * j + 1, j, C:128] = 1.0
        in_maps.append({
            "xT": xT, "wi": wi, "wa": wa, "we": we,
            "ones16": on, "sel": sel,
        })
    return in_maps


def run(x, weight_att, weight_input, weight_e, t_steps=T, trace=False):
    from concourse.bass_utils import run_bass_kernel_spmd

    nc = _build_nc(t_steps)
    in_maps = _host_prep(x, weight_att, weight_input, weight_e)
    if t_steps != T:
        for m in in_maps:
            m["xT"] = np.ascontiguousarray(m["xT"][:, :t_steps, :])
    res = run_bass_kernel_spmd(nc, in_maps, list(range(NCORES)), trace=trace)

    # results[g]["out"]: [t_steps, HL, B] -> out[b, t, g*HL + i]
    out = np.empty((B, t_steps, H), np.float32)
    for g in range(NCORES):
        og = res.results[g]["out"]
        out[:, :, g * HL:(g + 1) * HL] = og.transpose(2, 0, 1)
    return out, res


def kernel(x, weight_att, weight_input, weight_e):
    out, _ = run(x, weight_att, weight_input, weight_e)
    return out



# revision 6
# speedup vs baseline: 1.5215x; 1.5215x over previous
"""Trainium2 Bass kernel for nn_Attention_80805514707533.

Recurrent attention scan: B=512, T=512, C=64, H=128.
Sharding: H across 8 cores (16 heads each); full batch B=512 rides the
matmul moving dimension. C=64 lives on partitions; heads are packed in
pairs (2 x 64 = 128 partitions) with block-diagonal stationary weights.

v2 schedule: pairs processed in 4 groups of 2; per group one PSUM work
tile [128,2,512] is used in place for pre -> e (tanh evacuates pre,
the We matmul overwrites it).  All matmul operands are bf16 (FWL fast
weight loads, fp32 PSUM accumulate).  S and num share one PSUM bank
via full-width [128,16] ones stationaries.  Softmax scale broadcast
(bc) is a K=16 selector matmul into bf16 [128,2,512] PSUM tiles, then
one fused DVE multiply per group forms the next att state.

Per step t (per core, group g of pairs {2g, 2g+1}):
  pre[g]  = Wi_cat.T @ x_t  +  Wa_blk.T @ att[g]       (PE, PSUM)
  v[g]    = tanh(pre[g])                               (ACT -> bf16)
  e[g]    = We_blk.T @ v[g]      (in-place over pre)   (PE, PSUM)
  u[g]    = exp(e[g])                                  (ACT -> bf16)
  w[g]    = u[g] * xdup                                (DVE bf16 2x)
  S,num  += ones16 matmuls over u / w  -> [32, B]      (PE)
  rS      = approx 1/S                                 (DVE)
  out_t   = num * rS  -> DMA to DRAM                   (DVE)
  bc[g]   = sel.T @ rS  (bf16 PSUM)                    (PE)
  att'[g] = u[g] * bc[g]                               (DVE)
"""

import numpy as np
import ml_dtypes

B, T, C, H = 512, 512, 64, 128
NCORES = 8
HL = H // NCORES          # heads per core = 16
NPAIR = HL // 2           # head pairs per core = 8
NG = 4                    # groups per step
PPG = NPAIR // NG         # pairs per group = 2
TCH = 16                  # time steps per x chunk DMA

BF16 = ml_dtypes.bfloat16


def _build_nc(t_steps: int):
    import concourse.bass as bass
    import concourse.bacc as bacc
    import concourse.mybir as mybir
    import concourse.tile as tile
    from contextlib import ExitStack

    fp32 = mybir.dt.float32
    bf16 = mybir.dt.bfloat16
    fp32r = mybir.dt.float32r
    r = lambda ap: ap.bitcast(fp32r)
    nc = bacc.Bacc("TRN2", target_bir_lowering=False, debug=False,
                   num_devices=NCORES)

    n_ch = (t_steps + TCH - 1) // TCH
    xT_d = nc.dram_tensor("xT", [C, t_steps, B], bf16, kind="ExternalInput")
    wi_d = nc.dram_tensor("wi", [C, NPAIR, 128], bf16, kind="ExternalInput")
    wa_d = nc.dram_tensor("wa", [128, NPAIR, 128], bf16, kind="ExternalInput")
    we_d = nc.dram_tensor("we", [128, NPAIR, 128], bf16, kind="ExternalInput")
    on_d = nc.dram_tensor("ones16", [128, NPAIR, 16], bf16, kind="ExternalInput")
    sel_d = nc.dram_tensor("sel", [HL, NPAIR, 128], bf16, kind="ExternalInput")
    out_d = nc.dram_tensor("out", [t_steps, HL, B], fp32, kind="ExternalOutput")

    with ExitStack() as ctx:
        ctx.enter_context(nc.allow_low_precision(reason="bf16 matmul path"))
        tc = ctx.enter_context(tile.TileContext(nc))
        singles = ctx.enter_context(tc.tile_pool(name="singles", bufs=1))
        xpool = ctx.enter_context(tc.tile_pool(name="xpool", bufs=2))
        state = ctx.enter_context(tc.tile_pool(name="state", bufs=2))
        vpool = ctx.enter_context(tc.tile_pool(name="vpool", bufs=2))
        upool = ctx.enter_context(tc.tile_pool(name="upool", bufs=6))
        wpool = ctx.enter_context(tc.tile_pool(name="wpool", bufs=2))
        spool = ctx.enter_context(tc.tile_pool(name="spool", bufs=2))
        opool = ctx.enter_context(tc.tile_pool(name="opool", bufs=3))
        work = ctx.enter_context(tc.tile_pool(name="work", bufs=2, space="PSUM"))
        snp = ctx.enter_context(tc.tile_pool(name="snp", bufs=1, space="PSUM"))
        nmp = ctx.enter_context(tc.tile_pool(name="nmp", bufs=1, space="PSUM"))
        bcp = ctx.enter_context(tc.tile_pool(name="bcp", bufs=2, space="PSUM"))

        wi_sb = singles.tile([C, NPAIR, 128], bf16)
        wa_sb = singles.tile([128, NPAIR, 128], bf16)
        we_sb = singles.tile([128, NPAIR, 128], bf16)
        on_sb = singles.tile([128, NPAIR, 16], bf16)
        sel_sb = singles.tile([HL, NPAIR, 128], bf16)
        nc.sync.dma_start(out=wi_sb, in_=wi_d[:])
        nc.sync.dma_start(out=wa_sb, in_=wa_d[:])
        nc.sync.dma_start(out=we_sb, in_=we_d[:])
        nc.sync.dma_start(out=on_sb, in_=on_d[:])
        nc.sync.dma_start(out=sel_sb, in_=sel_d[:])

        att = state.tile([128, NPAIR, B], bf16, tag="att")
        nc.vector.memset(att, 1.0 / C)

        xch = None
        for t in range(t_steps):
            tl = t % TCH
            if tl == 0:
                nt = min(TCH, t_steps - t)
                xch = xpool.tile([128, TCH, B], bf16, tag="xch")
                nc.sync.dma_start(out=xch[0:C, 0:nt, :], in_=xT_d[:, t:t + nt, :])
                nc.sync.dma_start(out=xch[C:128, 0:nt, :], in_=xT_d[:, t:t + nt, :])
            xt = xch[:, tl, :]

            sn = snp.tile([HL, B], fp32, tag="sn")
            nm = nmp.tile([HL, B], fp32, tag="nm")
            u_tiles = []
            for g in range(NG):
                wk = work.tile([128, PPG, B], fp32, tag="work")
                for p in range(PPG):
                    j = g * PPG + p
                    nc.tensor.matmul(wk[:, p, :], wi_sb[:, j, :], xt[0:C, :],
                                     start=True, stop=False)
                    nc.tensor.matmul(wk[:, p, :], wa_sb[:, j, :], att[:, j, :],
                                     start=False, stop=True)
                v = vpool.tile([128, PPG, B], bf16, tag="v")
                nc.scalar.activation(v, wk, mybir.ActivationFunctionType.Tanh)
                for p in range(PPG):
                    j = g * PPG + p
                    nc.tensor.matmul(wk[:, p, :], we_sb[:, j, :], v[:, p, :],
                                     start=True, stop=True)
                u = upool.tile([128, PPG, B], bf16, tag="u")
                nc.scalar.activation(u, wk, mybir.ActivationFunctionType.Exp)
                u_tiles.append(u)
                w = wpool.tile([128, PPG, B], bf16, tag="w")
                for p in range(PPG):
                    j = g * PPG + p
                    nc.vector.tensor_mul(w[:, p, :], u[:, p, :], xt)
                    nc.tensor.matmul(sn, on_sb[:, j, :], u[:, p, :],
                                     start=(j == 0), stop=(j == NPAIR - 1),
                                     skip_group_check=True)
                    nc.tensor.matmul(nm, on_sb[:, j, :], w[:, p, :],
                                     start=(j == 0), stop=(j == NPAIR - 1),
                                     skip_group_check=True)

            rS = spool.tile([HL, B], fp32, tag="rS")
            nc.vector.reciprocal_approx_fast(out=rS, in_=sn)
            rSb = spool.tile([HL, B], bf16, tag="rSb")
            nc.vector.tensor_copy(rSb, rS)
            outb = opool.tile([HL, B], fp32, tag="outb")
            nc.vector.tensor_mul(outb, nm, rS)
            nc.sync.dma_start(out=out_d[t], in_=outb)

            att_new = state.tile([128, NPAIR, B], bf16, tag="att")
            for j in range(NPAIR):
                bc = bcp.tile([128, B], fp32, tag="bc")
                nc.tensor.matmul(bc, sel_sb[:, j, :], rSb,
                                 start=True, stop=True)
                nc.vector.tensor_mul(att_new[:, j, :],
                                     u_tiles[j // PPG][:, j % PPG, :], bc)
            att = att_new

    nc.compile()
    return nc


def _host_prep(x, weight_att, weight_input, weight_e):
    """Build per-core input maps (host-side layout prep)."""
    xT = np.ascontiguousarray(x.transpose(2, 1, 0)).astype(BF16)  # [C, T, B]

    in_maps = []
    for gcore in range(NCORES):
        h0 = gcore * HL
        wi = np.zeros((C, NPAIR, 128), np.float32)
        wa = np.zeros((128, NPAIR, 128), np.float32)
        we = np.zeros((128, NPAIR, 128), np.float32)
        on = np.zeros((128, NPAIR, 16), np.float32)
        sel = np.zeros((HL, NPAIR, 128), np.float32)
        for j in range(NPAIR):
            ha, hb = h0 + 2 * j, h0 + 2 * j + 1
            # lhsT[k, m] = W[h, m, k]
            wi[:, j, 0:C] = weight_input[ha].T
            wi[:, j, C:128] = weight_input[hb].T
            wa[0:C, j, 0:C] = weight_att[ha].T
            wa[C:128, j, C:128] = weight_att[hb].T
            we[0:C, j, 0:C] = weight_e[ha].T
            we[C:128, j, C:128] = weight_e[hb].T
            on[0:C, j, 2 * j] = 1.0
            on[C:128, j, 2 * j + 1] = 1.0
            sel[2 * j, j, 0:C] = 1.0
            sel[2 * j + 1, j, C:128] = 1.0
        in_maps.append({
            "xT": xT,
            "wi": wi.astype(BF16), "wa": wa.astype(BF16),
            "we": we.astype(BF16), "ones16": on.astype(BF16),
            "sel": sel.astype(BF16),
        })
    return in_maps


def run(x, weight_att, weight_input, weight_e, t_steps=T, trace=False):
    from concourse.bass_utils import run_bass_kernel_spmd

    nc = _build_nc(t_steps)
    in_maps = _host_prep(x, weight_att, weight_input, weight_e)
    if t_steps != T:
        for m in in_maps:
            m["xT"] = np.ascontiguousarray(m["xT"][:, :t_steps, :])
    res = run_bass_kernel_spmd(nc, in_maps, list(range(NCORES)), trace=trace)

    # results[g]["out"]: [t_steps, HL, B] -> out[b, t, g*HL + i]
    out = np.empty((B, t_steps, H), np.float32)
    for g in range(NCORES):
        og = res.results[g]["out"]
        out[:, :, g * HL:(g + 1) * HL] = og.transpose(2, 0, 1)
    return out, res


def kernel(x, weight_att, weight_input, weight_e):
    out, _ = run(x, weight_att, weight_input, weight_e)
    return out
